# revision 1
# baseline (speedup 1.0000x reference)
"""Trainium2 Bass kernel for causal multi-head attention (B=2, T=4096, C=768, H=12).

Sharding: 8 cores = 2 batches x 4 head-groups (3 heads each).
Each core computes, for its batch b and heads hg = [3g, 3g+3):
    qkv = x[b] @ Wqkv[:, local cols]      (Q pre-scaled by 1/sqrt(C))
    per head: scoresT[k, q] = K^T-layout matmuls, exp, causal mask,
    row sums via an appended ones-column on V, yT = V_aug^T @ exp_sT,
    normalized by the sums, then out_partial = y_local @ Wout[local rows].
Host sums the 4 partial outputs per batch.

Everything on-chip is laid out transposed (feature dim on partitions) so no
transposes are ever needed: scores come out as [k_chunk=128, q=512] tiles,
softmax sums ride along as row 64 of the attnV PSUM accumulator.

All matmul operands are float32r (TF32-like, 1 cycle/row at N>=256,
~1e-4 matmul relative error). PSUM accumulation is fp32.
"""

import numpy as np

import concourse.bass as bass
import concourse.mybir as mybir
import concourse.tile as tile
from concourse import bacc
from concourse.bass_utils import run_bass_kernel_spmd

dt = mybir.dt

B, T, C, H = 2, 4096, 768, 12
D = C // H                  # 64
HEADS_PER_CORE = 3
N_CORES = 8
CCHUNKS = C // 128          # 6 contraction chunks for the projections
QT = 512                    # q tile (moving dim)
NQT = T // QT               # 8
KC = 128                    # k chunk (scores partition dim)
CLOC = HEADS_PER_CORE * D   # 192 local channels
WV_PAD = 256                # V-projection moving dim padded so f32r runs 1cyc/row

_CACHE = {}
_SPLIT_WAITS = False  # HW-verified unnecessary; kept as a safety valve


def _split_matmul_waits(nc):
    """Fused-weight-load (fp32/f32r) matmuls encode as S3_LW, which walrus
    only allows ONE sync wait on. bacc's generate_event_semaphores leaves up
    to two. Hoist all but one wait onto an InstEventSemaphore right before
    the matmul on the same engine queue."""
    n_split = 0
    for f in nc.m.functions:
        for blk in f.blocks:
            out = []
            changed = False
            for ins in blk.instructions:
                if isinstance(ins, mybir.InstMatmult):
                    si = ins.sync_info
                    waits = list(si.on_wait) if si is not None else []
                    if len(waits) > 1:
                        extra = waits[:-1]
                        for i in range(0, len(extra), 2):
                            ev = mybir.InstEventSemaphore(
                                name=f"{ins.name}-wsplit{i}", ins=[], outs=[])
                            ev.engine = ins.engine
                            ev.sync_info = mybir.SyncInfo(
                                on_wait=extra[i:i + 2], on_update=[])
                            nc.register_instruction(ev)
                            out.append(ev)
                        ins.sync_info = mybir.SyncInfo(
                            on_wait=[waits[-1]], on_update=list(si.on_update))
                        n_split += 1
                        changed = True
                out.append(ins)
            if changed:
                blk.instructions = out
    return n_split


def _build(T=T, stage="full"):
    NQT = T // QT
    nc = bacc.Bacc("TRN2", target_bir_lowering=False, debug=False)

    xT = nc.dram_tensor("xT", [C, T], dt.float32r, kind="ExternalInput").ap()
    wqk = nc.dram_tensor("wqk", [128, CCHUNKS * 2 * CLOC], dt.float32r,
                         kind="ExternalInput").ap()
    wv = nc.dram_tensor("wv", [128, CCHUNKS * WV_PAD], dt.float32r,
                        kind="ExternalInput").ap()
    wout = nc.dram_tensor("wout", [128, 2 * C], dt.float32r,
                          kind="ExternalInput").ap()
    masks = nc.dram_tensor("masks", [128, 4 * QT], dt.float32,
                           kind="ExternalInput").ap()
    ones = nc.dram_tensor("ones", [128, 64], dt.float32r,
                          kind="ExternalInput").ap()
    out = nc.dram_tensor("out", [T, C], dt.float32, kind="ExternalOutput").ap()

    with tile.TileContext(nc) as tc:
        with tc.tile_pool(name="const", bufs=1) as cpool:
            w_qk = cpool.tile([128, CCHUNKS, 2 * CLOC], dt.float32r)
            w_v = cpool.tile([128, CCHUNKS, WV_PAD], dt.float32r)
            w_out = cpool.tile([128, 2, C], dt.float32r)
            msk = cpool.tile([128, 4, QT], dt.float32)
            one = cpool.tile([128, 64], dt.float32r)
            nc.gpsimd.dma_start(out=w_qk[:, :, :], in_=wqk[:, :])
            nc.gpsimd.dma_start(out=w_v[:, :, :], in_=wv[:, :])
            nc.gpsimd.dma_start(out=w_out[:, :, :], in_=wout[:, :])
            nc.gpsimd.dma_start(out=msk[:, :, :], in_=masks[:, :])
            nc.gpsimd.dma_start(out=one[:, :], in_=ones[:, :])

            # Persistent activations. [64, T] tensors are packed in pairs so
            # every scores matmul has lhsT/rhs at the SAME partition base
            # (hardware requirement):
            #   t_q01: Q0 | Q1      t_k01: K0 | K1      (h0 -> base 0, h1 -> base 64)
            #   t_q2y: Q2 | yT2     t_k2y: K2 | yT0     (h2 -> base 0)
            #   t_y1:  yT1 | -
            t_q01 = cpool.tile([128, T], dt.float32r)
            t_k01 = cpool.tile([128, T], dt.float32r)
            t_q2y = cpool.tile([128, T], dt.float32r)
            t_k2y = cpool.tile([128, T], dt.float32r)
            t_y1 = cpool.tile([128, T], dt.float32r)
            q_sb = [t_q01[0:64], t_q01[64:128], t_q2y[0:64]]
            k_sb = [t_k01[0:64], t_k01[64:128], t_k2y[0:64]]
            # yT0|yT1 stacked in one tile (single K=128 stage-C matmul);
            # yT2 reuses Q2's partitions (each Q slice is dead after its own
            # q-tile's scores matmuls -- WAR deps keep this safe).
            y_sb = [t_y1[0:64], t_y1[64:128], t_q2y[0:64]]

            v_sb = [cpool.tile([128, T // 128, D + 1], dt.float32r,
                               name=f"v{h}", tag=f"v{h}")
                    for h in range(HEADS_PER_CORE)]
            for h in range(HEADS_PER_CORE):
                nc.vector.memset(v_sb[h].bitcast(dt.uint32)[:, :, D:D + 1],
                                 0x3F800000)  # 1.0f ones column for row sums

            # ------- Pipelined: projections + attention + out-proj -------
            heads = list(range(HEADS_PER_CORE)) if stage != "a" else []
            do_c = stage == "full"
            with (
                tc.tile_pool(name="xs", bufs=2 * CCHUNKS) as xs_pool,
                tc.tile_pool(name="ex", bufs=6) as ex_pool,
                tc.tile_pool(name="nrm", bufs=4) as nrm_pool,
                tc.tile_pool(name="ps_pa", bufs=1, space="PSUM") as ps_pa,
                tc.tile_pool(name="ps_x", bufs=1, space="PSUM") as ps_x,
                tc.tile_pool(name="ps_s", bufs=2, space="PSUM") as ps_s,
                tc.tile_pool(name="ps_y", bufs=2, space="PSUM") as ps_y,
            ):
                def stage_a(t):
                    ts = slice(t * QT, (t + 1) * QT)
                    xt = []
                    for c in range(CCHUNKS):
                        xc = xs_pool.tile([128, QT], dt.float32r,
                                          name="xt", tag="xt")
                        nc.gpsimd.dma_start(
                            out=xc[:, :], in_=xT[c * 128:(c + 1) * 128, ts])
                        xt.append(xc)
                    for h in range(HEADS_PER_CORE):
                        pa = ps_pa.tile([128, QT], dt.float32, name="pa", tag="pa")
                        for c in range(CCHUNKS):
                            nc.tensor.matmul(
                                out=pa[:, :],
                                lhsT=w_qk[:, c, h * 128:(h + 1) * 128],
                                rhs=xt[c][:, :],
                                start=(c == 0), stop=(c == CCHUNKS - 1))
                        nc.vector.tensor_copy(out=q_sb[h][:, ts], in_=pa[0:64, :])
                        nc.vector.tensor_copy(out=k_sb[h][:, ts], in_=pa[64:128, :])
                    for s in range(QT // 128):
                        pv = ps_x.tile([128, QT], dt.float32, name="pv", tag="x")
                        for c in range(CCHUNKS):
                            nc.tensor.matmul(
                                out=pv[:, 0:WV_PAD],
                                lhsT=xt[c][:, s * 128:(s + 1) * 128],
                                rhs=w_v[:, c, :],
                                start=(c == 0), stop=(c == CCHUNKS - 1))
                        j = t * (QT // 128) + s
                        for h in range(HEADS_PER_CORE):
                            nc.vector.tensor_copy(
                                out=v_sb[h][:, j, 0:D],
                                in_=pv[:, h * D:(h + 1) * D])

                def attn_pair(h, qt, pi, py, nchunks):
                    qs = slice(qt * QT, (qt + 1) * QT)
                    ps = ps_s.tile([128, 2 * QT], dt.float32, name="ps", tag="ps")
                    for j2 in range(2):
                        kc = 2 * pi + j2
                        nc.tensor.matmul(
                            out=ps[:, j2 * QT:(j2 + 1) * QT],
                            lhsT=k_sb[h][:, kc * KC:(kc + 1) * KC],
                            rhs=q_sb[h][:, qs],
                            start=True, stop=True)
                    et = ex_pool.tile([128, 2 * QT], dt.float32r,
                                      name="et", tag="et")
                    nc.scalar.activation(
                        out=et[:, :], in_=ps[:, :],
                        func=mybir.ActivationFunctionType.Exp)
                    for j2 in range(2):
                        kc = 2 * pi + j2
                        r = kc - qt * (QT // KC)
                        if r >= 0:
                            nc.vector.tensor_mul(
                                out=et[:, j2 * QT:(j2 + 1) * QT],
                                in0=et[:, j2 * QT:(j2 + 1) * QT],
                                in1=msk[:, r, :])
                    for j2 in range(2):
                        kc = 2 * pi + j2
                        nc.tensor.matmul(
                            out=py[:, :],
                            lhsT=v_sb[h][:, kc, :],
                            rhs=et[:, j2 * QT:(j2 + 1) * QT],
                            start=(kc == 0), stop=(kc == nchunks - 1))

                def attn_normalize(h, qt, py):
                    qs = slice(qt * QT, (qt + 1) * QT)
                    sums = nrm_pool.tile([128, QT], dt.float32r,
                                         name="sums", tag="sums")
                    nc.vector.tensor_copy(out=sums[64:65, :], in_=py[D:D + 1, :])
                    pr = ps_x.tile([64, QT], dt.float32, name="pr", tag="x")
                    nc.tensor.matmul(out=pr[:, :], lhsT=one[64:65, :],
                                     rhs=sums[64:65, :], start=True, stop=True)
                    recip = nrm_pool.tile([64, QT], dt.float32,
                                          name="recip", tag="recip")
                    nc.vector.reciprocal(out=recip[:, :], in_=pr[:, :])
                    nc.vector.tensor_mul(out=y_sb[h][:, qs],
                                         in0=py[0:D, :], in1=recip[:, :])

                for t in range(NQT):
                    stage_a(t)
                    qt = t
                    nchunks = (qt + 1) * (QT // KC)
                    for h in heads:
                        py = ps_y.tile([D + 1, QT], dt.float32,
                                       name="py", tag="py")
                        for pi in range(nchunks // 2):
                            attn_pair(h, qt, pi, py, nchunks)
                        attn_normalize(h, qt, py)

            if do_c:
                with (
                    tc.tile_pool(name="oc", bufs=3) as oc_pool,
                    tc.tile_pool(name="ps_c", bufs=3, space="PSUM") as ps_c,
                ):
                    for t in range(T // 128):
                        ts = slice(t * 128, (t + 1) * 128)
                        ot = oc_pool.tile([128, C], dt.float32,
                                          name="ot", tag="ot")
                        for n0 in range(0, C, 512):
                            n1 = min(n0 + 512, C)
                            pc = ps_c.tile([128, 512], dt.float32,
                                           name="pc", tag="pc")
                            nc.tensor.matmul(
                                out=pc[:, 0:n1 - n0], lhsT=t_y1[:, ts],
                                rhs=w_out[:, 0, n0:n1], start=True, stop=False)
                            nc.tensor.matmul(
                                out=pc[:, 0:n1 - n0], lhsT=y_sb[2][:, ts],
                                rhs=w_out[0:64, 1, n0:n1], start=False, stop=True)
                            if n0 == 0:
                                nc.vector.tensor_copy(out=ot[:, n0:n1],
                                                      in_=pc[:, 0:n1 - n0])
                            else:
                                nc.scalar.copy(out=ot[:, n0:n1],
                                               in_=pc[:, 0:n1 - n0])
                        nc.sync.dma_start(out=out[ts, :], in_=ot[:, :])

            if stage == "a":
                with tc.tile_pool(name="oca", bufs=2) as oca_pool:
                    for t in range(T // 128):
                        ts = slice(t * 128, (t + 1) * 128)
                        ot = oca_pool.tile([128, C], dt.float32, tag="ota")
                        nc.vector.memset(ot[:, :], 0.0)
                        nc.vector.tensor_copy(out=ot[:, 0:128], in_=t_q01[:, t * 128:(t + 1) * 128])
                        nc.vector.tensor_copy(out=ot[:, 128:256], in_=t_k01[:, t * 128:(t + 1) * 128])
                        nc.vector.tensor_copy(out=ot[:, 256:384], in_=t_q2y[:, t * 128:(t + 1) * 128])
                        nc.vector.tensor_copy(out=ot[:, 384:512], in_=t_k2y[:, t * 128:(t + 1) * 128])
                        nc.sync.dma_start(out=out[ts, :], in_=ot[:, :])
            if stage == "attn":
                with tc.tile_pool(name="ocd", bufs=2) as ocd_pool:
                    for t in range(T // 128):
                        ts = slice(t * 128, (t + 1) * 128)
                        ot = ocd_pool.tile([128, C], dt.float32, tag="otd")
                        nc.vector.memset(ot[:, :], 0.0)
                        nc.vector.tensor_copy(out=ot[0:64, 0:128], in_=y_sb[0][:, t * 128:(t + 1) * 128])
                        nc.vector.tensor_copy(out=ot[0:64, 128:256], in_=y_sb[1][:, t * 128:(t + 1) * 128])
                        nc.vector.tensor_copy(out=ot[0:64, 256:384], in_=y_sb[2][:, t * 128:(t + 1) * 128])
                        nc.sync.dma_start(out=out[ts, :], in_=ot[:, :])

    nc.compile()
    if _SPLIT_WAITS:
        _split_matmul_waits(nc)
    return nc


def _host_inputs(x, W_qkv, W_out):
    """Per-core input maps. Core order: core = 4*b + g."""
    x = np.asarray(x, dtype=np.float32)
    W_qkv = np.asarray(W_qkv, dtype=np.float32)
    W_out = np.asarray(W_out, dtype=np.float32)
    scale = 1.0 / np.sqrt(np.float32(C))

    mask = np.zeros((128, 4, QT), dtype=np.float32)
    p = np.arange(128)[:, None]
    j = np.arange(QT)[None, :]
    for r in range(4):
        mask[:, r, :] = (j >= p + 128 * r).astype(np.float32)
    mask = np.ascontiguousarray(mask.reshape(128, 4 * QT))
    ones = np.ones((128, 64), dtype=np.float32)

    in_maps = []
    for core in range(N_CORES):
        b, g = divmod(core, 4)
        heads = range(HEADS_PER_CORE * g, HEADS_PER_CORE * (g + 1))
        xTb = np.ascontiguousarray(x[b].T)  # [C, T]

        # wqk: [128, 6, 384]; per head slot h: cols [h*128, h*128+64) = Q_h
        # (pre-scaled), [h*128+64, (h+1)*128) = K_h
        wqk = np.zeros((CCHUNKS, 128, 2 * CLOC), dtype=np.float32)
        wv = np.zeros((CCHUNKS, 128, WV_PAD), dtype=np.float32)
        for i, hh in enumerate(heads):
            q_col = W_qkv[:, hh * D:(hh + 1) * D] * scale
            k_col = W_qkv[:, C + hh * D:C + (hh + 1) * D]
            v_col = W_qkv[:, 2 * C + hh * D:2 * C + (hh + 1) * D]
            wqk[:, :, i * 128:i * 128 + D] = q_col.reshape(CCHUNKS, 128, D)
            wqk[:, :, i * 128 + D:(i + 1) * 128] = k_col.reshape(CCHUNKS, 128, D)
            wv[:, :, i * D:(i + 1) * D] = v_col.reshape(CCHUNKS, 128, D)
        wqk = np.ascontiguousarray(
            wqk.transpose(1, 0, 2).reshape(128, CCHUNKS * 2 * CLOC))
        wv = np.ascontiguousarray(
            wv.transpose(1, 0, 2).reshape(128, CCHUNKS * WV_PAD))

        # wout: [128, 2, 768]: slot 0 = rows for heads 0,1 stacked (K=128
        # stage-C matmul), slot 1 top half = head 2 rows
        hh = list(heads)
        wo = np.zeros((128, 2, C), dtype=np.float32)
        wo[0:64, 0, :] = W_out[hh[0] * D:(hh[0] + 1) * D, :]
        wo[64:128, 0, :] = W_out[hh[1] * D:(hh[1] + 1) * D, :]
        wo[0:64, 1, :] = W_out[hh[2] * D:(hh[2] + 1) * D, :]
        wo = np.ascontiguousarray(wo.reshape(128, 2 * C))

        in_maps.append({
            "xT": xTb, "wqk": wqk, "wv": wv, "wout": wo,
            "masks": mask, "ones": ones,
        })
    return in_maps


def get_nc(T_arg=T, stage="full"):
    key = ("nc", T_arg, stage)
    if key not in _CACHE:
        _CACHE[key] = _build(T_arg, stage)
    return _CACHE[key]


def kernel(x, W_qkv, W_out):
    nc = get_nc()
    in_maps = _host_inputs(x, W_qkv, W_out)
    res = run_bass_kernel_spmd(nc, in_maps, list(range(N_CORES)))
    out = np.zeros((B, T, C), dtype=np.float32)
    for core in range(N_CORES):
        b = core // 4
        out[b] += res.results[core]["out"]
    return out

